# revision 7
# baseline (speedup 1.0000x reference)
"""Causal self-attention (B=4, S=2048, E=1024, H=16) on 8 trn2 NeuronCores.

Sharding: data parallel over batch (4) x tensor parallel over head groups (2).
Core c handles batch c//2, heads [ (c%2)*8, (c%2)*8+8 ).  Each core computes
its group's QKV projections, causal attention, and a partial output
projection; the host sums the two group partials per batch and adds bo.

All matmuls run in float32r (fp32 storage, single-pass relaxed-precision PE
mode — full bf16-rate at free dim >= 256, ~2^-10 product precision), with
fp32 PSUM accumulation.  Measured end-to-end error is ~10x tighter than a
bf16 datapath.

Device layout notes:
  - x arrives pre-transposed from host: xT [E, S], streamed per 512-token
    chunk.  QT/KT are produced feature-major [c, s] (lhsT = W, rhs = xT);
    V token-major [s, f] (lhsT = xT, rhs = Wv).
  - scores are computed transposed [k, q]: lhsT = KT head slice [64, 128],
    rhs = QT head slice [64, 512]; head parity selects partition base 0/64.
  - softmax denominator comes from a 65th ones-column in the AV stationary
    operand: PSUM row 64 of the [65, q] attention-value output is the
    running row-sum of exp(scores).  It is broadcast to 64 partitions with
    a K=1 matmul against a ones vector.
  - causal masking: fully-masked k-tiles are skipped; on diagonal tiles the
    dead columns are memset to zero and the 128-wide triangle multiplied in.
"""

import numpy as np

import concourse.mybir as mybir
import concourse.tile as tile
from concourse import bacc
from concourse.bass_utils import run_bass_kernel_spmd

F32 = mybir.dt.float32
F32R = mybir.dt.float32r
Exp = mybir.ActivationFunctionType.Exp
MULT = mybir.AluOpType.mult
ADD = mybir.AluOpType.add

B, S, E, H = 4, 2048, 1024, 16
D = 64          # head dim
HG = 8          # heads per core
G = 512         # group feature width
P = 128
NKT = S // P    # 16 k-tiles
NST = S // P    # 16 s-tiles
QB = 512        # q-block width
NQB = S // QB   # 4
ESUB = E // P   # 8
VW = D + 1      # V stationary width (64 dims + ones column)

_CACHE = {}


def _build_program():
    nc = bacc.Bacc("TRN2", target_bir_lowering=False, debug=False)

    xt_d = nc.dram_tensor("xt", [E, S], F32R, kind="ExternalInput").ap()
    wq_d = nc.dram_tensor("wq", [E, G], F32R, kind="ExternalInput").ap()
    wk_d = nc.dram_tensor("wk", [E, G], F32R, kind="ExternalInput").ap()
    wv_d = nc.dram_tensor("wv", [E, G], F32R, kind="ExternalInput").ap()
    wo_d = nc.dram_tensor("wo", [G, E], F32R, kind="ExternalInput").ap()
    bq_d = nc.dram_tensor("bq", [P, 4], F32, kind="ExternalInput").ap()
    bk_d = nc.dram_tensor("bk", [P, 4], F32, kind="ExternalInput").ap()
    bv_d = nc.dram_tensor("bv", [P, G], F32, kind="ExternalInput").ap()
    tri_d = nc.dram_tensor("tri", [P, P], F32R, kind="ExternalInput").ap()
    one_d = nc.dram_tensor("one", [P, D], F32R, kind="ExternalInput").ap()
    out_d = nc.dram_tensor("out", [S, E], F32, kind="ExternalOutput").ap()

    qt_sb = nc.alloc_sbuf_tensor("qt_sb", [P, 4, S], F32R).ap()
    kt_sb = nc.alloc_sbuf_tensor("kt_sb", [P, 4, S], F32R).ap()
    vx_sb = nc.alloc_sbuf_tensor("vx_sb", [P, NKT, HG, VW], F32R).ap()
    tri_sb = nc.alloc_sbuf_tensor("tri_sb", [P, P], F32R).ap()
    ones_sb = nc.alloc_sbuf_tensor("ones_sb", [P, D], F32R).ap()
    bq_sb = nc.alloc_sbuf_tensor("bq_sb", [P, 4], F32).ap()
    bk_sb = nc.alloc_sbuf_tensor("bk_sb", [P, 4], F32).ap()
    bv_sb = nc.alloc_sbuf_tensor("bv_sb", [P, G], F32).ap()

    with tile.TileContext(nc) as tc:
        nc.sync.dma_start(bq_sb[:], bq_d[:])
        nc.sync.dma_start(bk_sb[:], bk_d[:])
        nc.sync.dma_start(bv_sb[:], bv_d[:])
        nc.sync.dma_start(tri_sb[:], tri_d[:])
        nc.sync.dma_start(ones_sb[:], one_d[:])

        xt_r = xt_d.rearrange("(o p) s -> p o s", p=P)

        # ---- QKV projections (xT streamed per 512-token chunk) ----
        with (
            tc.tile_pool(name="w_pool", bufs=1) as wp,
            tc.tile_pool(name="xt_pool", bufs=2) as xp,
            tc.tile_pool(name="proj_ps", bufs=2, space="PSUM") as pp,
        ):
            wqp = wp.tile([P, ESUB, G], F32R, tag="wq", name="wqp")
            wkp = wp.tile([P, ESUB, G], F32R, tag="wk", name="wkp")
            wvp = wp.tile([P, ESUB, G], F32R, tag="wv", name="wvp")
            nc.sync.dma_start(wqp[:], wq_d.rearrange("(o p) f -> p o f", p=P))
            nc.sync.dma_start(wkp[:], wk_d.rearrange("(o p) f -> p o f", p=P))
            nc.sync.dma_start(wvp[:], wv_d.rearrange("(o p) f -> p o f", p=P))
            for qb in range(NQB):
                xtp = xp.tile([P, ESUB, QB], F32R, tag="xt", name="xtp")
                nc.sync.dma_start(xtp[:], xt_r[:, :, qb * QB : (qb + 1) * QB])
                for cc in range(4):
                    q_ps = pp.tile([P, QB], F32, tag="q", name="q_ps")
                    for e in range(ESUB):
                        nc.tensor.matmul(
                            q_ps[:],
                            lhsT=wqp[:, e, cc * P : (cc + 1) * P],
                            rhs=xtp[:, e, :],
                            start=(e == 0),
                            stop=(e == ESUB - 1),
                        )
                    # fold bias and the 1/sqrt(D) scale into Q
                    nc.vector.tensor_scalar(
                        qt_sb[:, cc, qb * QB : (qb + 1) * QB],
                        q_ps[:],
                        bq_sb[:, cc : cc + 1],
                        0.125,
                        ADD,
                        MULT,
                    )
                    k_ps = pp.tile([P, QB], F32, tag="k", name="k_ps")
                    for e in range(ESUB):
                        nc.tensor.matmul(
                            k_ps[:],
                            lhsT=wkp[:, e, cc * P : (cc + 1) * P],
                            rhs=xtp[:, e, :],
                            start=(e == 0),
                            stop=(e == ESUB - 1),
                        )
                    nc.vector.tensor_scalar_add(
                        kt_sb[:, cc, qb * QB : (qb + 1) * QB],
                        k_ps[:],
                        bk_sb[:, cc : cc + 1],
                    )
                for stl in range(4):
                    st = qb * 4 + stl
                    v_ps = pp.tile([P, G], F32, tag="v", name="v_ps")
                    for e in range(ESUB):
                        nc.tensor.matmul(
                            v_ps[:],
                            lhsT=xtp[:, e, stl * P : (stl + 1) * P],
                            rhs=wvp[:, e, :],
                            start=(e == 0),
                            stop=(e == ESUB - 1),
                        )
                    nc.vector.tensor_tensor(
                        vx_sb[:, st, :, 0:D],
                        v_ps.rearrange("p (h d) -> p h d", d=D),
                        bv_sb.rearrange("p (h d) -> p h d", d=D),
                        ADD,
                    )
                    nc.vector.tensor_copy(
                        vx_sb[:, st, :, D : D + 1],
                        ones_sb[:, 0:HG].rearrange("p (h u) -> p h u", u=1),
                    )

        # ---- attention + output projection ----
        with tc.tile_pool(name="at_pool", bufs=1) as atp:
            at_t = atp.tile([P, 4, S], F32R, name="at_t")
            with (
                tc.tile_pool(name="attn_ps", bufs=1, space="PSUM") as ap,
                tc.tile_pool(name="attn_sb", bufs=2) as sp,
            ):
                for h in range(HG):
                    sub = h // 2
                    hb = (h % 2) * D
                    for qp in range(2):
                        qb_lo, qb_hi = 2 * qp, 2 * qp + 1
                        a_ps = ap.tile([VW, 2, QB], F32, tag="a", name="a_ps")
                        for kt in range(4 * qb_hi + 4):
                            qlo = max(qb_lo, kt // 4)
                            j0 = qlo - qb_lo
                            s_ps = ap.tile(
                                [P, 2, QB], F32, tag="s", name="s_ps", bufs=2
                            )
                            for qb in range(qlo, qb_hi + 1):
                                j = qb - qb_lo
                                nc.tensor.matmul(
                                    s_ps[:, j, :],
                                    lhsT=kt_sb[hb : hb + D, sub, kt * P : (kt + 1) * P],
                                    rhs=qt_sb[hb : hb + D, sub, qb * QB : (qb + 1) * QB],
                                    start=True,
                                    stop=True,
                                )
                            pt = sp.tile([P, 2, QB], F32R, tag="pt", name="pt", bufs=3)
                            nc.scalar.activation(pt[:, j0:2, :], s_ps[:, j0:2, :], Exp)
                            if kt // 4 == qlo:
                                m = kt % 4
                                if m > 0:
                                    nc.vector.tensor_scalar_mul(
                                        pt[:, j0, 0 : P * m], pt[:, j0, 0 : P * m], 0.0
                                    )
                                nc.vector.tensor_tensor(
                                    pt[:, j0, P * m : P * (m + 1)],
                                    pt[:, j0, P * m : P * (m + 1)],
                                    tri_sb[:],
                                    MULT,
                                )
                            for qb in range(qlo, qb_hi + 1):
                                j = qb - qb_lo
                                nc.tensor.matmul(
                                    a_ps[:, j, :],
                                    lhsT=vx_sb[:, kt, h, :],
                                    rhs=pt[:, j, :],
                                    start=(kt == 0),
                                    stop=(kt == 4 * qb + 3),
                                )
                        # normalize; PSUM row 64 holds the denominator
                        rs = sp.tile([VW, 2, QB], F32R, tag="rs", name="rs", bufs=2)
                        with nc.allow_low_precision(
                            reason="f32r is full fp32 storage; PE rounding only"
                        ):
                            nc.vector.reciprocal(rs[D:VW, :, :], a_ps[D:VW, :, :])
                        rb_ps = ap.tile([D, 2, QB], F32, tag="rb", name="rb_ps")
                        for j in range(2):
                            nc.tensor.matmul(
                                rb_ps[:, j, :],
                                lhsT=ones_sb[D : D + 1, :],
                                rhs=rs[D:VW, j, :],
                                start=True,
                                stop=True,
                            )
                        rb_sb = sp.tile([D, 2, QB], F32, tag="rbs", name="rb_sb", bufs=2)
                        nc.vector.tensor_copy(rb_sb[:], rb_ps[:])
                        at_slice = at_t[hb : hb + D, sub, qp * 1024 : (qp + 1) * 1024]
                        at_slice = at_slice.rearrange("p (a b) -> p a b", b=QB)
                        if hb == 0:
                            nc.vector.tensor_tensor(
                                at_slice, a_ps[0:D, :, :], rb_sb[:], MULT
                            )
                        else:
                            tmp = sp.tile([D, 2, QB], F32R, tag="tmp", name="tmp", bufs=2)
                            nc.vector.tensor_tensor(tmp[:], a_ps[0:D, :, :], rb_sb[:], MULT)
                            nc.sync.dma_start(at_slice, tmp[:])

            # ---- output projection (partial; host adds other group + bo) ----
            with (
                tc.tile_pool(name="wo_pool", bufs=1) as wop_pool,
                tc.tile_pool(name="op_ps", bufs=2, space="PSUM") as op,
                tc.tile_pool(name="op_sb", bufs=3) as osp,
            ):
                wop = wop_pool.tile([P, 4, E], F32R, name="wop")
                nc.sync.dma_start(wop[:], wo_d.rearrange("(o p) n -> p o n", p=P))
                for st in range(NST):
                    for n in range(2):
                        o_ps = op.tile([P, QB], F32, tag="o", name="o_ps")
                        for t in range(4):
                            nc.tensor.matmul(
                                o_ps[:],
                                lhsT=at_t[:, t, st * P : (st + 1) * P],
                                rhs=wop[:, t, n * QB : (n + 1) * QB],
                                start=(t == 0),
                                stop=(t == 3),
                            )
                        o_sb = osp.tile([P, QB], F32, tag="ost", name="o_sb")
                        nc.vector.tensor_copy(o_sb[:], o_ps[:])
                        nc.sync.dma_start(
                            out_d[st * P : (st + 1) * P, n * QB : (n + 1) * QB],
                            o_sb[:],
                        )

    nc.compile()
    return nc


def _prep_inputs(x, Wqkv, bqkv, Wo, bo):
    x = np.asarray(x, np.float32)
    Wqkv = np.asarray(Wqkv, np.float32)
    bqkv = np.asarray(bqkv, np.float32)
    Wo = np.asarray(Wo, np.float32)

    # 128x128 inclusive lower-triangle-in-(q,k) == kl <= ql in [k, q] layout
    kl = np.arange(P)[:, None]
    ql = np.arange(P)[None, :]
    tri = (kl <= ql).astype(np.float32)

    in_maps = []
    for c in range(8):
        b, g = divmod(c, 2)
        lo, hi = g * G, (g + 1) * G
        in_maps.append(
            {
                "xt": np.ascontiguousarray(x[b].T),
                "wq": np.ascontiguousarray(Wqkv[:, lo:hi]),
                "wk": np.ascontiguousarray(Wqkv[:, E + lo : E + hi]),
                "wv": np.ascontiguousarray(Wqkv[:, 2 * E + lo : 2 * E + hi]),
                "wo": np.ascontiguousarray(Wo[lo:hi, :]),
                "bq": np.ascontiguousarray(bqkv[lo:hi].reshape(4, P).T),
                "bk": np.ascontiguousarray(bqkv[E + lo : E + hi].reshape(4, P).T),
                "bv": np.tile(bqkv[2 * E + lo : 2 * E + hi][None, :], (P, 1)).astype(
                    np.float32
                ),
                "tri": tri,
                "one": np.ones((P, D), np.float32),
            }
        )
    return in_maps


def kernel(x, Wqkv, bqkv, Wo, bo, _trace=False):
    if "nc" not in _CACHE:
        _CACHE["nc"] = _build_program()
    nc = _CACHE["nc"]

    in_maps = _prep_inputs(x, Wqkv, bqkv, Wo, bo)
    res = run_bass_kernel_spmd(nc, in_maps, core_ids=list(range(8)), trace=_trace)
    _CACHE["last_result"] = res

    bo = np.asarray(bo, np.float32)
    out = np.empty((B, S, E), np.float32)
    for b in range(B):
        out[b] = res.results[2 * b]["out"] + res.results[2 * b + 1]["out"] + bo
    return out
